# revision 42
# baseline (speedup 1.0000x reference)
"""ChunkCrossAttention Trainium2 kernel (v2: fp8 + AllGather-KV).

Math (per reference):
  x = chunk_embeddings[0]                      # (S, L)
  k, v = split(x @ W_kv.T)                     # (S, D) each
  scores = einsum('jqd,sd->jqs', q, k) / sqrt(D), masked
  attn = softmax(scores, -1)
  out = (attn @ v) @ W_out.T + q  -> LayerNorm(gamma, beta)

Strategy (8 NeuronCores):
  - Queries sharded: each core owns 1024 q rows end-to-end (no partial
    softmax, no ReduceScatter of 8MB partials like v1).
  - W_out folded into the value projection on the host (Wvo = W_out @ Wv),
    so phase 1 emits k^T [d, s] and v' [s, d] directly in the layouts the
    attention matmuls need.
  - All matmuls fp8(e4m3) DoubleRow: 2x bf16 PE throughput. Weights are
    prescaled x64 on host (e4m3 min-normal 2^-6 vs W ~ N(0, 1/64^2));
    the psum->fp8 copies divide back by 64.
  - KV projection sharded over S (512 keys/core), then the tiny fp8 KV
    blob (264KB/core) is AllGather'd once (91 GB/s at this size) and
    unpacked per block across three DMA queues. (The cc stream costs
    ~20us init + ~11us/op fixed + a 30-55us cross-core start-skew
    barrier that no kernel structure can remove; phase 1 and the input
    streams hide under it.)
  - Softmax without max subtraction, with a global shift exp(x-3)
    (softmax-invariant) to keep fp8 exponents in range. exp alternates
    per key-subtile between Act (native Exp -> fp8) and DVE
    (Schraudolph: construct the e4m3 bit pattern directly as
    round(score*8*SCALE*log2e + const) with a saturating uint8 convert;
    negatives clamp to 0x00=+0.0, masked keys get bias -1e9 -> 0).
    One merged op covers both q-halves of a subtile (same keys, same
    mask bias).
  - Denominator via a ones column appended to v' (DP=260 wide attn out).
  - LN scale invariance: LN(num/den + q) == LN(num + den*q), so the
    epilogue needs no reciprocal and no division.
  - Attention from staged exp tiles: the scores->exp stream stages all
    softmax weights to SBUF (2 double-bank score slots); replay units
    for q-tiles 0-3 (4 matmuls each, no exp dependency) stream 3 groups
    behind it so the PE never idles on exp latency; q-tiles 4-7 replay
    afterwards on the freed score banks, each q-tile's LN epilogue
    overlapping the next q-tile's matmuls.
  - gamma/beta are applied only when not identity (checked at runtime;
    the program variant is JIT-selected and cached).
"""
import sys

sys.path.insert(0, "/opt/trn_rl_repo")

import numpy as np

import concourse.bacc as bacc
import concourse.mybir as mybir
import concourse.tile as tile
from concourse.bass_utils import run_bass_kernel_spmd

N_CORES = 8
J, Q, D = 64, 128, 256
S, L = 4096, 4096
S_LOC = S // N_CORES          # 512 keys per core
QR = (J // N_CORES) * Q       # 1024 query rows per core
DP = D + 4                    # attn free: 256 outputs + denom + 3 pad
LN_EPS = 1e-5
SCALE = 1.0 / np.sqrt(D)
SHIFT = 3.0                   # global exp shift (softmax invariant)
LOG2E = 1.4426950408889634
A_CONST = 8.0 * SCALE * LOG2E
BITS_B = 8.0 * (7.0 - SHIFT * LOG2E) - 0.5
WPRE = 64.0                   # host weight prescale before fp8 cast

F32 = mybir.dt.float32
F16 = mybir.dt.float16
FP8 = mybir.dt.float8e4
U8 = mybir.dt.uint8
AF = mybir.ActivationFunctionType
ALU = mybir.AluOpType
PM = mybir.MatmulPerfMode

N_LB = L // 256               # 16 L-pairs (256 contraction rows each)


def build_program(apply_gb=True):
    nc = bacc.Bacc(None, num_devices=N_CORES)

    xT8 = nc.declare_dram_parameter("xT8", [L, S_LOC], FP8, isOutput=False)
    wT8 = nc.declare_dram_parameter("wT8", [L, 2 * D], FP8, isOutput=False)
    qT8 = nc.declare_dram_parameter("qT8", [2 * 128, QR], FP8, isOutput=False)
    qres = nc.declare_dram_parameter("qres", [QR, D], F16, isOutput=False)
    dve_b = nc.declare_dram_parameter("dve_b", [128, 32], F32, isOutput=False)
    act_b = nc.declare_dram_parameter("act_b", [128, 32], F32, isOutput=False)
    gamma = nc.declare_dram_parameter("gamma", [D], F32, isOutput=False)
    beta = nc.declare_dram_parameter("beta", [D], F32, isOutput=False)
    y = nc.declare_dram_parameter("y", [QR, D], F16, isOutput=True)

    ag_in = nc.dram_tensor("ag_in", [128, 2064], FP8)
    ag_out = nc.dram_tensor("ag_out", [N_CORES, 128, 2064], FP8,
                            addr_space="Shared")

    import concourse.bass as bass

    with tile.TileContext(nc) as tc:
        with tc.tile_pool(name="singles", bufs=1) as singles, \
             tc.tile_pool(name="wpool", bufs=1) as wpool, \
             tc.tile_pool(name="xpool", bufs=N_LB + 4) as xpool, \
             tc.tile_pool(name="ex1", bufs=1) as ex1p, \
             tc.tile_pool(name="hp", bufs=6) as hpool, \
             tc.tile_pool(name="small", bufs=24) as small:

            # ---- persistent loads (scalar DMA queue; sync/vector carry
            # the phase-1 streams) ----
            qT_sb = singles.tile([128, 2, QR], FP8)
            nc.gpsimd.dma_start(out=qT_sb,
                                in_=qT8.rearrange("(c p) q -> p c q", p=128))
            dve_b_sb = singles.tile([128, 32], F32)
            nc.gpsimd.dma_start(out=dve_b_sb, in_=dve_b[:, :])
            act_b_sb = singles.tile([128, 32], F32)
            nc.gpsimd.dma_start(out=act_b_sb, in_=act_b[:, :])
            qres_sb = singles.tile([128, QR // 128, D], F16)
            for qq in range(2):
                nc.gpsimd.dma_start(
                    out=qres_sb[:, qq * 4:(qq + 1) * 4, :],
                    in_=qres.rearrange("(t p) d -> p t d", p=128)[
                        :, qq * 4:(qq + 1) * 4, :])
            g_ap = gamma[:]
            gamma_sb = singles.tile([128, D], F32)
            nc.gpsimd.dma_start(out=gamma_sb, in_=bass.AP(
                tensor=g_ap.tensor, offset=g_ap.offset,
                ap=[[0, 128], g_ap.ap[0]]))
            b_ap = beta[:]
            beta_sb = singles.tile([128, D], F32)
            nc.gpsimd.dma_start(out=beta_sb, in_=bass.AP(
                tensor=b_ap.tensor, offset=b_ap.offset,
                ap=[[0, 128], b_ap.ap[0]]))
            eps_sb = singles.tile([128, 1], F32)
            nc.vector.memset(eps_sb, LN_EPS)

            kT_loc = singles.tile([128, 2, S_LOC], FP8)
            vp_loc = singles.tile([128, 4, DP], FP8)
            nc.vector.memset(vp_loc[:, :, D:D + 1], 1.0)
            nc.vector.memset(vp_loc[:, :, D + 1:DP], 0.0)
            kT_all = singles.tile([128, N_CORES, 2, S_LOC], FP8)
            vp_all = singles.tile([128, N_CORES, 4, DP], FP8)

            # ---- phase 1: local KV projection (fp8 DR), AG per quarter ----
            ps1 = tc.tile_pool(name="ps1", bufs=1, space="PSUM")
            ps_1 = ps1.__enter__()
            kacc = [ps_1.tile([128, 2, 256], F32, name=f"kacc{h}")
                    for h in range(2)]
            vacc = [ps_1.tile([128, 256], F32, name=f"vacc{qd}")
                    for qd in range(4)]

            wts = []
            for lb in range(N_LB):
                wt = wpool.tile([128, 2, 2 * D], FP8, tag=f"wt{lb}",
                                name=f"wt{lb}")
                nc.sync.dma_start(
                    out=wt,
                    in_=wT8[lb * 256:(lb + 1) * 256, :].rearrange(
                        "(a p) n -> p a n", p=128))
                wts.append(wt)

            def kick_ag():
                # blob [128, 2064]: [0:1024]=kT (dc-major, contiguous),
                # [1024:2064]=vp (quarter-major). Both sides of each unpack
                # are then contiguous per core, so the whole gather is TWO
                # DMA triggers (trigger setup costs ~0.6-0.9us each on the
                # queue engine - 24 small unpacks serialized ~11us).
                for dc in range(2):
                    nc.gpsimd.dma_start(
                        out=ag_in[:, dc * 512:(dc + 1) * 512],
                        in_=kT_loc[:, dc, :])
                nc.gpsimd.dma_start(out=ag_in[:, 1024:2064],
                                    in_=vp_loc[:, :, :])
                nc.gpsimd.collective_compute(
                    "AllGather", ALU.bypass,
                    replica_groups=[list(range(N_CORES))],
                    ins=[ag_in[:, :]], outs=[ag_out[:, :, :]])
                gathered = ag_out.rearrange("c p f -> p c f")
                nc.sync.dma_start(out=kT_all[:, 0:4, :, :],
                                  in_=gathered[:, 0:4, 0:1024])
                nc.gpsimd.dma_start(out=kT_all[:, 4:8, :, :],
                                    in_=gathered[:, 4:8, 0:1024])
                nc.scalar.dma_start(out=vp_all[:, :, :, :],
                                    in_=gathered[:, :, 1024:2064])

            for h in range(2):
                xts = []
                for xc in range(N_LB // 2):
                    xt = xpool.tile([128, 2, 2, 256], FP8, tag="xt")
                    nc.scalar.dma_start(
                        out=xt,
                        in_=xT8[xc * 512:(xc + 1) * 512,
                                h * 256:(h + 1) * 256].rearrange(
                                    "(l a p) k -> p l a k", p=128, a=2))
                    xts.append(xt)
                for lb in range(N_LB):
                    xt_s = xts[lb // 2][:, lb % 2, :, :]
                    for kd in range(2):
                        nc.tensor.matmul(
                            kacc[h][:, kd, :],
                            wts[lb][:, :, kd * 128:(kd + 1) * 128],
                            xt_s,
                            start=(lb == 0), stop=(lb == N_LB - 1),
                            perf_mode=PM.DoubleRow)
                nc.scalar.activation(out=kT_loc[:, :, h * 256:(h + 1) * 256],
                                     in_=kacc[h], func=AF.Copy,
                                     scale=1.0 / WPRE)
                for qq in range(2):
                    qd = 2 * h + qq
                    for lb in range(N_LB):
                        nc.tensor.matmul(
                            vacc[qd],
                            xts[lb // 2][:, lb % 2, :,
                                         qq * 128:(qq + 1) * 128],
                            wts[lb][:, :, D:2 * D],
                            start=(lb == 0), stop=(lb == N_LB - 1),
                            perf_mode=PM.DoubleRow)
                    nc.scalar.activation(out=vp_loc[:, qd, 0:D], in_=vacc[qd],
                                         func=AF.Copy, scale=1.0 / WPRE)
            kick_ag()
            ps1.__exit__(None, None, None)

            # ---- phase 2a: scores -> exp stream, fully staged to SBUF.
            # During the second pair-column's scores, the pc0 attention
            # replay of q-tiles 0-3 interleaves on the PE (it has no exp
            # dependency, so it fills every exp-latency bubble). ----
            ps2e = tc.tile_pool(name="ps_e", bufs=1, space="PSUM")
            ps_e = ps2e.__enter__()
            at03 = [ps_e.tile([128, DP], F32, name=f"at{qt}")
                    for qt in range(4)]
            ps2s = tc.tile_pool(name="ps_sc", bufs=2, space="PSUM")
            ps_sc = ps2s.__enter__()

            # staged exp tiles, [128, sub, qh, 512]
            ex1_tiles = {}
            for b in range(N_CORES):
                for pc in range(2):
                    ex1_tiles[(b, pc)] = ex1p.tile(
                        [128, 2, 2, 512], FP8, name=f"ex1_{b}_{pc}")

            # exp engine split: alternate Act / DVE per key-subtile so
            # every q row averages both engines' quantization error. One
            # merged op covers both q-halves of a key-subtile (same keys
            # -> same mask bias). (gpsimd cannot read PSUM.)
            # 9:7 Act:DVE split (Act's merged op is ~20% faster)
            EXP_ENG = "ADADADAADADADAAD"

            def do_exp(ext_slice, sc, b, st):
                key_idx = 4 * b + st
                if EXP_ENG[key_idx % 16] == "A":
                    nc.scalar.activation(out=ext_slice, in_=sc, func=AF.Exp,
                                         bias=act_b_sb[:, key_idx:key_idx + 1],
                                         scale=SCALE)
                else:
                    nc.vector.tensor_scalar(out=ext_slice.bitcast(U8), in0=sc,
                                            scalar1=A_CONST,
                                            scalar2=dve_b_sb[:, key_idx:key_idx + 1],
                                            op0=ALU.mult, op1=ALU.add)

            # ---- epilogue: h' = num + den*q, LN (scale-invariant) ----
            y_r = y.rearrange("(t p) d -> t p d", p=128)

            def epilogue(qt, acc):
                den = small.tile([128, 1], F32, tag="den")
                nc.vector.tensor_copy(out=den, in_=acc[:, D:D + 1])
                h = hpool.tile([128, D], F32, tag="h")
                nc.scalar.activation(out=h, in_=qres_sb[:, qt, :],
                                     func=AF.Copy, scale=den)
                nc.vector.tensor_add(out=h, in0=h, in1=acc[:, 0:D])
                stats = small.tile([128, 6], F32, tag="stats")
                nc.vector.bn_stats(out=stats, in_=h)
                mv = small.tile([128, 2], F32, tag="mv")
                nc.vector.bn_aggr(out=mv, in_=stats)
                rstd = small.tile([128, 1], F32, tag="rstd")
                nc.scalar.activation(out=rstd, in_=mv[:, 1:2], func=AF.Sqrt,
                                     bias=eps_sb, scale=1.0)
                nc.vector.reciprocal(out=rstd, in_=rstd)
                nmr = small.tile([128, 1], F32, tag="nmr")
                nc.vector.tensor_scalar(out=nmr, in0=mv[:, 0:1],
                                        scalar1=rstd, scalar2=-1.0,
                                        op0=ALU.mult, op1=ALU.mult)
                if apply_gb:
                    xh = hpool.tile([128, D], F32, tag="xh")
                    nc.scalar.activation(out=xh, in_=h, func=AF.Identity,
                                         bias=nmr, scale=rstd)
                    yt = hpool.tile([128, D], F16, tag="yt")
                    ge = nc.gpsimd if qt % 2 == 0 else nc.vector
                    ge.tensor_mul(out=yt, in0=xh, in1=gamma_sb)
                    ge.tensor_add(out=yt, in0=yt, in1=beta_sb)
                else:
                    # gamma==1, beta==0 (checked at runtime): the affine
                    # is the identity, write the normalized row directly
                    yt = hpool.tile([128, D], F16, tag="yt")
                    nc.scalar.activation(out=yt, in_=h, func=AF.Identity,
                                         bias=nmr, scale=rstd)
                [nc.sync, nc.gpsimd][qt % 2].dma_start(
                    out=y_r[qt], in_=yt)

            def scores_group(pc, b):
                ext = ex1_tiles[(b, pc)]
                for sub in range(2):
                    st = 2 * pc + sub
                    sc = ps_sc.tile([128, 2, 512], F32, tag="sc")
                    for qh in range(2):
                        nc.tensor.matmul(
                            sc[:, qh, :],
                            kT_all[:, b, :, st * 128:(st + 1) * 128],
                            qT_sb[:, :, qh * 512:(qh + 1) * 512],
                            start=True, stop=True,
                            perf_mode=PM.DoubleRow)
                    do_exp(ext[:, sub, :, :], sc, b, st)

            def replay_part(acc, qt, pc, blocks, start, stop):
                qh, qi = (0, qt) if qt < 4 else (1, qt - 4)
                for i, bb in enumerate(blocks):
                    nc.tensor.matmul(
                        acc,
                        ex1_tiles[(bb, pc)][:, :, qh,
                                            qi * 128:(qi + 1) * 128],
                        vp_all[:, bb, 2 * pc:2 * pc + 2, :],
                        start=(start and i == 0),
                        stop=(stop and i == len(blocks) - 1),
                        perf_mode=PM.DoubleRow)

            # stream replay units (4 matmuls, one per q-tile 0-3) with a
            # 3-group lag behind the scores/exp stream: PE work per group
            # (4 scores + 4 replay) stays above the exp-engine pace, so
            # the PE never idles on exp latency in either pair-column.
            def replay_unit(j, stop):
                pc, bb = j // 8, j % 8
                for qt in range(4):
                    replay_part(at03[qt], qt, pc, [bb],
                                start=(j == 0), stop=stop)

            LAG = 3
            for i in range(16):
                scores_group(i // 8, i % 8)
                if i >= LAG:
                    replay_unit(i - LAG, stop=False)
            for j in range(16 - LAG, 15):
                replay_unit(j, stop=False)
            replay_unit(15, stop=True)
            for qt in range(4):
                epilogue(qt, at03[qt])

            ps2s.__exit__(None, None, None)

            # ---- phase 2b: q-half 1 replay on the banks freed by the
            # score slots; each q-tile's epilogue overlaps the next
            # q-tile's matmuls. ----
            ps2d = tc.tile_pool(name="ps_d", bufs=4, space="PSUM")
            ps_d = ps2d.__enter__()
            for qt in range(4, 8):
                acc = ps_d.tile([128, DP], F32, tag="at_d", name="at_d")
                replay_part(acc, qt, 0, range(N_CORES), start=True,
                            stop=False)
                replay_part(acc, qt, 1, range(N_CORES), start=False,
                            stop=True)
                epilogue(qt, acc)

            ps2d.__exit__(None, None, None)
            ps2e.__exit__(None, None, None)

    nc.finalize()
    return nc


_NC_CACHE = {}


def _make_in_maps(inputs):
    import ml_dtypes
    e4 = ml_dtypes.float8_e4m3

    def q8(a):
        return np.clip(a, -240.0, 240.0).astype(e4)

    jq = np.asarray(inputs["justice_queries"], dtype=np.float32)
    x = np.asarray(inputs["chunk_embeddings"], dtype=np.float32)[0]
    mask = np.asarray(inputs["chunk_mask"])
    wkv = np.asarray(inputs["W_kv"], dtype=np.float32)
    wout = np.asarray(inputs["W_out"], dtype=np.float32)
    gamma = np.asarray(inputs["ln_gamma"], dtype=np.float32)
    beta = np.asarray(inputs["ln_beta"], dtype=np.float32)

    wk = wkv[:D]
    wvo = wout @ wkv[D:]                          # fold W_out into Wv
    wT = np.concatenate([wk, wvo], axis=0) * WPRE  # (512, L)
    wT8 = np.ascontiguousarray(q8(wT.T))           # (L, 512)
    xT8_full = np.ascontiguousarray(q8(x.T))       # (L, S)

    flat = np.ascontiguousarray(jq.reshape(J * Q, D))
    mask_on = mask != 0
    # per-key exp biases, laid out [p, b*4+st] for key = b*512+st*128+p
    dve_b = np.empty((128, 32), dtype=np.float32)
    act_b = np.empty((128, 32), dtype=np.float32)
    for col in range(32):
        b_, st = col // 4, col % 4
        keys = b_ * 512 + st * 128 + np.arange(128)
        on = mask_on[keys]
        dve_b[:, col] = np.where(on, BITS_B, -1e9)
        act_b[:, col] = np.where(on, -SHIFT, -1e30)

    in_maps = []
    for c in range(N_CORES):
        qrows = flat[c * QR:(c + 1) * QR]          # (1024, 256)
        qT = np.ascontiguousarray(qrows.T)         # (256, 1024)
        in_maps.append({
            "xT8": np.ascontiguousarray(
                xT8_full[:, c * S_LOC:(c + 1) * S_LOC]),
            "wT8": wT8,
            "qT8": np.ascontiguousarray(q8(qT)),
            "qres": np.ascontiguousarray(qrows.astype(np.float16)),
            "dve_b": dve_b,
            "act_b": act_b,
            "gamma": gamma,
            "beta": beta,
        })
    return in_maps


def kernel(**inputs) -> np.ndarray:
    in_maps = _make_in_maps(inputs)
    apply_gb = not (np.all(np.asarray(inputs["ln_gamma"]) == 1.0)
                    and np.all(np.asarray(inputs["ln_beta"]) == 0.0))
    if apply_gb not in _NC_CACHE:
        _NC_CACHE[apply_gb] = build_program(apply_gb)
    res = run_bass_kernel_spmd(_NC_CACHE[apply_gb], in_maps,
                               list(range(N_CORES)))
    out = np.concatenate([res.results[c]["y"] for c in range(N_CORES)], axis=0)
    return np.ascontiguousarray(out.reshape(J, Q, D).astype(np.float32))
